# revision 1
# baseline (speedup 1.0000x reference)
"""Trainium2 Bass kernel: dense transformer block (attention + per-batch bmm + FF).

Sharding: 8 cores = (batch b = c//2) x (query-half nh = c%2).
Each core computes attention for all 16 heads over its 1024 query rows
(keys/values over full S=2048, recomputed per batch-pair), then the
per-batch feature-reduction bmm and the feed-forward for its rows.

All heavy matmuls run as float32r (fp22 mantissa, full PE rate at free>=256).
Scores are computed twice: pass 1 (packed 2-head) feeds a VectorE running-max
+ GPSIMD partition-reduce to get the per-query row max; pass 2 re-computes
scores with a 65th contraction row (k_aug row = -1, q_aug row = +rowmax) so
the shift is baked into PSUM and ScalarE can do shift+exp+evacuate in one
activation pass. V carries a 65th ones-column so the softmax denominator
falls out of the attn@v matmul for free; z is normalized via a K=1
broadcast matmul of the reciprocal + one tensor_tensor multiply.
"""

import sys

sys.path.insert(0, "/opt/trn_rl_repo")

import numpy as np

B, S, E, H, HF = 4, 2048, 1024, 16, 64
NH = 1024          # query rows per core
NB = NH // 512     # 512-wide query blocks per core
SCALE = 1.0 / np.sqrt(HF)

_CACHE = {}
_CST = np.concatenate([np.ones((1, S), np.float32), -np.ones((1, S), np.float32)])


def _np_reference(x, attention_mask, Wq, Wk, Wv, Wr, Wff, bff):
    """Fallback (used only if the mask is not all-ones)."""
    x64 = x.astype(np.float64)
    q = np.einsum("bse,hef->bhsf", x64, Wq.astype(np.float64)).reshape(B * H, S, HF)
    k = np.einsum("bse,hef->bhsf", x64, Wk.astype(np.float64)).reshape(B * H, S, HF)
    v = np.einsum("bse,hef->bhsf", x64, Wv.astype(np.float64)).reshape(B * H, S, HF)
    s = np.matmul(q, k.transpose(0, 2, 1))
    s = np.where(attention_mask[0] == 0, -1e9, s)
    s = s * SCALE
    s = s - s.max(axis=-1, keepdims=True)
    p = np.exp(s)
    p /= p.sum(axis=-1, keepdims=True)
    z = np.matmul(p, v).reshape(B, H, S, HF).transpose(0, 2, 1, 3).reshape(B, S, E)
    z = np.matmul(z, Wr.astype(np.float64))
    o = np.maximum(z @ Wff.astype(np.float64).T + bff.astype(np.float64), 0.0)
    return o.astype(np.float32)


def _build():
    import concourse.bacc as bacc
    import concourse.bass as bass
    import concourse.mybir as mybir
    import concourse.tile as tile
    import bass_rust

    F32 = mybir.dt.float32
    BF16 = mybir.dt.bfloat16
    F32R = mybir.dt.float32r
    MULT = mybir.AluOpType.mult
    MAXOP = mybir.AluOpType.max
    EXP = mybir.ActivationFunctionType.Exp
    RELU = mybir.ActivationFunctionType.Relu
    RMAX = bass_rust.ReduceOp.max
    PSUM = bass.MemorySpace.PSUM

    def r(ap):
        return ap.bitcast(F32R)

    nc = bacc.Bacc("TRN2", target_bir_lowering=False, debug=False)
    xt_d = nc.dram_tensor("xt", [E, S], F32, kind="ExternalInput")      # x[b].T, cols rolled so my half is first
    wq_d = nc.dram_tensor("wq", [E, E], F32, kind="ExternalInput")      # [e, h*HF+f]
    wk_d = nc.dram_tensor("wk", [E, E], F32, kind="ExternalInput")
    wv_d = nc.dram_tensor("wv", [E, E], F32, kind="ExternalInput")
    wr_d = nc.dram_tensor("wr", [E, E], F32, kind="ExternalInput")      # Wr[b]
    wfft_d = nc.dram_tensor("wfft", [E, E], F32, kind="ExternalInput")  # Wff.T
    bff_d = nc.dram_tensor("bff", [E, 1], F32, kind="ExternalInput")
    cst_d = nc.dram_tensor("cst", [2, S], F32, kind="ExternalInput")   # rows: 1.0, -1.0
    out_d = nc.dram_tensor("o", [E, NH], F32, kind="ExternalOutput")    # [j, n]

    with tile.TileContext(nc) as tc:
        with tc.tile_pool(name="glob", bufs=1) as glob:
            zTn = glob.tile([128, 8, NH], F32)     # normalized z^T: [f-in-pair, echunk, n]
            bfft = glob.tile([128, 8], F32)
            nc.sync.dma_start(out=bfft, in_=bff_d.ap().rearrange("(t p) o -> p (t o)", p=128))
            ones65 = glob.tile([1, 65], F32)
            nc.sync.dma_start(out=r(ones65), in_=r(cst_d.ap())[0:1, 0:65])

            # ---------------- phase 1: projections + attention ----------------
            with tc.tile_pool(name="p1x", bufs=1) as p1x, \
                 tc.tile_pool(name="wpool", bufs=1) as wpool, \
                 tc.tile_pool(name="qkpool", bufs=1) as qkpool, \
                 tc.tile_pool(name="vpool", bufs=1) as vpool, \
                 tc.tile_pool(name="epool", bufs=3) as epool, \
                 tc.tile_pool(name="spool", bufs=2) as spool, \
                 tc.tile_pool(name="ps_a", bufs=2, space=PSUM) as ps_a, \
                 tc.tile_pool(name="ps_s2", bufs=1, space=PSUM) as ps_s2, \
                 tc.tile_pool(name="ps_z", bufs=2, space=PSUM) as ps_z:

                xt = p1x.tile([128, 8, S], F32)    # 64KB/part
                nc.sync.dma_start(out=r(xt), in_=r(xt_d.ap()).rearrange("(i p) m -> p i m", p=128))

                for hp in range(H // 2):           # head pair: heads 2hp, 2hp+1
                    h0, h1 = 2 * hp, 2 * hp + 1
                    # -------- R1: projections for this pair (v per 4-head group) --------
                    wq_sb = wpool.tile([128, 8, 128], F32, tag="wq")
                    nc.sync.dma_start(out=r(wq_sb), in_=r(wq_d.ap()).rearrange("(i p) c -> p i c", p=128)[:, :, hp * 128:(hp + 1) * 128])
                    wk_sb = wpool.tile([128, 8, 128], F32, tag="wk")
                    nc.sync.dma_start(out=r(wk_sb), in_=r(wk_d.ap()).rearrange("(i p) c -> p i c", p=128)[:, :, hp * 128:(hp + 1) * 128])

                    # qT / kT (f-major, 2 heads stacked) + aug copies per head
                    qT2 = qkpool.tile([128, NH], F32, tag="qT2")
                    qaug = [qkpool.tile([65, NH], F32, tag=f"qaug{i}", name=f"qaug{i}") for i in range(2)]
                    kT2 = qkpool.tile([128, S], F32, tag="kT2")
                    kaug = [qkpool.tile([65, S], F32, tag=f"kaug{i}", name=f"kaug{i}") for i in range(2)]
                    nc.sync.dma_start(out=r(kaug[0][64:65, :]), in_=r(cst_d.ap())[1:2, :])
                    nc.sync.dma_start(out=r(kaug[1][64:65, :]), in_=r(cst_d.ap())[1:2, :])

                    for nb in range(NB):
                        psq = ps_a.tile([128, 512], F32, tag="psa")
                        for e in range(8):
                            nc.tensor.matmul(psq, r(wq_sb[:, e, :]), r(xt[:, e, nb * 512:(nb + 1) * 512]),
                                             start=(e == 0), stop=(e == 7))
                        sl = slice(nb * 512, (nb + 1) * 512)
                        nc.vector.tensor_copy(out=r(qT2[:, sl]), in_=psq)
                        nc.sync.dma_start(out=r(qaug[0][0:64, sl]), in_=r(qT2[0:64, sl]))
                        nc.sync.dma_start(out=r(qaug[1][0:64, sl]), in_=r(qT2[64:128, sl]))
                    for mb in range(S // 512):
                        psk = ps_a.tile([128, 512], F32, tag="psa")
                        for e in range(8):
                            nc.tensor.matmul(psk, r(wk_sb[:, e, :]), r(xt[:, e, mb * 512:(mb + 1) * 512]),
                                             start=(e == 0), stop=(e == 7))
                        sl = slice(mb * 512, (mb + 1) * 512)
                        nc.vector.tensor_copy(out=r(kT2[:, sl]), in_=psk)
                        nc.sync.dma_start(out=r(kaug[0][0:64, sl]), in_=r(kT2[0:64, sl]))
                        nc.sync.dma_start(out=r(kaug[1][0:64, sl]), in_=r(kT2[64:128, sl]))

                    if hp % 2 == 0:                # v for 4 heads (free dim 256)
                        g = hp // 2
                        wv_sb = wpool.tile([128, 8, 256], F32, tag="wv")
                        nc.sync.dma_start(out=r(wv_sb), in_=r(wv_d.ap()).rearrange("(i p) c -> p i c", p=128)[:, :, g * 256:(g + 1) * 256])
                        v4 = vpool.tile([128, 16, 4, 65], BF16, tag="v4")
                        nc.vector.memset(v4[:, :, :, 64:65], 1.0)
                        for mt in range(16):
                            psv = ps_a.tile([128, 512], F32, tag="psa")
                            for e in range(8):
                                nc.tensor.matmul(psv[:, 0:256], r(xt[:, e, mt * 128:(mt + 1) * 128]), r(wv_sb[:, e, :]),
                                                 start=(e == 0), stop=(e == 7))
                            nc.scalar.copy(out=v4[:, mt, :, 0:64], in_=psv[:, 0:256].rearrange("p (s f) -> p s f", s=4))

                    # -------- per nb: pass-1 max scan, then pass-2 + av (interleaves on engines) --------
                    for nb in range(NB):
                        macc = [spool.tile([128, 512], F32, tag=f"macc{i}", name=f"macc{i}", bufs=2) for i in range(2)]
                        for mt in range(16):
                            ps1 = [ps_a.tile([128, 512], F32, tag="psa", name=f"ps1_{_i}") for _i in range(2)]
                            msl = slice(mt * 128, (mt + 1) * 128)
                            nsl = slice(nb * 512, (nb + 1) * 512)
                            nc.tensor.matmul(ps1[0], r(kT2[0:64, msl]), r(qT2[0:64, nsl]), start=True, stop=True)
                            nc.tensor.matmul(ps1[1], r(kT2[64:128, msl]), r(qT2[64:128, nsl]), start=True, stop=True)
                            for i in range(2):
                                if mt == 0:
                                    nc.vector.tensor_copy(out=macc[i], in_=ps1[i])
                                else:
                                    nc.vector.tensor_tensor(out=macc[i], in0=ps1[i], in1=macc[i], op=MAXOP)
                        for i in range(2):
                            mall = spool.tile([128, 512], F32, tag=f"mall{i}", bufs=2)
                            nc.gpsimd.partition_all_reduce(mall, macc[i], channels=128, reduce_op=RMAX)
                            nc.sync.dma_start(out=r(qaug[i][64:65, nb * 512:(nb + 1) * 512]), in_=r(mall[0:1, :]))

                        for i, h in enumerate((h0, h1)):
                            ka, qa = kaug[i], qaug[i]
                            nsl = slice(nb * 512, (nb + 1) * 512)
                            zps = ps_z.tile([128, 512], F32, tag="zps")
                            first_done = False
                            for mm in range(4):            # macro-tile of 4 m-tiles
                                ps2 = ps_s2.tile([128, 4, 512], F32, tag="ps2")
                                for mi in range(4):
                                    mt = mm * 4 + mi
                                    nc.tensor.matmul(ps2[:, mi, :], r(ka[0:65, mt * 128:(mt + 1) * 128]), r(qa[0:65, nsl]),
                                                     start=True, stop=True)
                                expt = epool.tile([128, 4, 512], BF16, tag="expt")
                                nc.scalar.activation(out=expt, in_=ps2, func=EXP, scale=float(SCALE))
                                for mi in range(4):
                                    mt = mm * 4 + mi
                                    g, s4 = h // 4, h % 4
                                    nc.tensor.matmul(zps[0:65, :], v4[:, mt, s4, :], expt[:, mi, :],
                                                     start=(not first_done), stop=(mt == 15))
                                    first_done = True
                            # normalize: recip(denom) broadcast via K=1 matmul, then TT mult
                            den65 = spool.tile([65, 512], F32, tag="den65")
                            nc.vector.tensor_copy(out=den65[64:65, :], in_=zps[64:65, :])
                            den = spool.tile([1, 512], F32, tag="den")
                            nc.sync.dma_start(out=den, in_=den65[64:65, :])
                            rec = spool.tile([1, 512], F32, tag="rec")
                            with nc.allow_low_precision(reason="recip read as fp32r by PE broadcast"):
                                nc.vector.reciprocal(out=r(rec), in_=den)
                            rps = ps_z.tile([128, 512], F32, tag="zps")
                            nc.tensor.matmul(rps[0:65, :], r(ones65), r(rec), start=True, stop=True)
                            recb = spool.tile([65, 512], F32, tag="recb")
                            nc.vector.tensor_copy(out=recb, in_=rps[0:65, :])
                            ec = h // 2
                            if h % 2 == 0:
                                nc.vector.tensor_tensor(out=r(zTn[0:64, ec, nsl]), in0=zps[0:64, :], in1=recb[0:64, :], op=MULT)
                            else:
                                ztmp = spool.tile([64, 512], F32, tag="ztmp")
                                nc.vector.tensor_tensor(out=r(ztmp), in0=zps[0:64, :], in1=recb[0:64, :], op=MULT)
                                nc.sync.dma_start(out=r(zTn[64:128, ec, nsl]), in_=r(ztmp))

            # ---------------- phase 2: y^T = Wr^T-style bmm, then FF ----------------
            with tc.tile_pool(name="p2w", bufs=1) as p2w, \
                 tc.tile_pool(name="p2y", bufs=1) as p2y, \
                 tc.tile_pool(name="p2o", bufs=2) as p2o, \
                 tc.tile_pool(name="ps_y", bufs=2, space=PSUM) as ps_y:
                wr_sb = p2w.tile([128, 8, E], F32, tag="wr")
                nc.sync.dma_start(out=r(wr_sb), in_=r(wr_d.ap()).rearrange("(i p) c -> p i c", p=128))
                wff_sb = p2w.tile([128, 8, E], F32, tag="wff")
                nc.sync.dma_start(out=r(wff_sb), in_=r(wfft_d.ap()).rearrange("(i p) c -> p i c", p=128))
                yT = p2y.tile([128, 8, NH], F32)
                for dt in range(8):
                    psy = ps_y.tile([128, NH], F32, tag="psy")
                    for ec in range(8):
                        for half in range(2):
                            nc.tensor.matmul(psy[:, half * 512:(half + 1) * 512],
                                             r(wr_sb[:, ec, dt * 128:(dt + 1) * 128]),
                                             r(zTn[:, ec, half * 512:(half + 1) * 512]),
                                             start=(ec == 0), stop=(ec == 7))
                    nc.vector.tensor_copy(out=r(yT[:, dt, :]), in_=psy)
                for jt in range(8):
                    pso = ps_y.tile([128, NH], F32, tag="psy")
                    for dc in range(8):
                        for half in range(2):
                            nc.tensor.matmul(pso[:, half * 512:(half + 1) * 512],
                                             r(wff_sb[:, dc, jt * 128:(jt + 1) * 128]),
                                             r(yT[:, dc, half * 512:(half + 1) * 512]),
                                             start=(dc == 0), stop=(dc == 7))
                    ot = p2o.tile([128, NH], F32, tag="ot")
                    nc.scalar.activation(out=ot, in_=pso, func=RELU, bias=bfft[:, jt:jt + 1], scale=1.0)
                    nc.sync.dma_start(out=out_d.ap()[jt * 128:(jt + 1) * 128, :], in_=ot)

    nc.compile()
    return nc


def _get_module():
    if "nc" not in _CACHE:
        _CACHE["nc"] = _build()
    return _CACHE["nc"]


def kernel(x, attention_mask, Wq, Wk, Wv, Wr, Wff, bff):
    from concourse import bass_utils

    x = np.asarray(x, dtype=np.float32)
    attention_mask = np.asarray(attention_mask)
    Wq = np.asarray(Wq, dtype=np.float32)
    Wk = np.asarray(Wk, dtype=np.float32)
    Wv = np.asarray(Wv, dtype=np.float32)
    Wr = np.asarray(Wr, dtype=np.float32)
    Wff = np.asarray(Wff, dtype=np.float32)
    bff = np.asarray(bff, dtype=np.float32)

    if not np.all(attention_mask == 1):
        return _np_reference(x, attention_mask, Wq, Wk, Wv, Wr, Wff, bff)

    nc = _get_module()
    wq2 = np.ascontiguousarray(Wq.transpose(1, 0, 2).reshape(E, E))
    wk2 = np.ascontiguousarray(Wk.transpose(1, 0, 2).reshape(E, E))
    wv2 = np.ascontiguousarray(Wv.transpose(1, 0, 2).reshape(E, E))
    wfft = np.ascontiguousarray(Wff.T)
    bff2 = np.ascontiguousarray(bff.reshape(E, 1))

    in_maps = []
    for c in range(8):
        b, nh = c // 2, c % 2
        xt = x[b].T
        if nh:
            xt = np.concatenate([xt[:, NH:], xt[:, :NH]], axis=1)
        in_maps.append({
            "xt": np.ascontiguousarray(xt),
            "wq": wq2, "wk": wk2, "wv": wv2,
            "wr": np.ascontiguousarray(Wr[b]),
            "wfft": wfft, "bff": bff2,
            "cst": _CST,
        })

    res = bass_utils.run_bass_kernel_spmd(nc, in_maps, core_ids=list(range(8)), **_CACHE.get("run_kwargs", {}))
    _CACHE["last_result"] = res

    out = np.empty((B, S, E), dtype=np.float32)
    for c in range(8):
        b, nh = c // 2, c % 2
        out[b, nh * NH:(nh + 1) * NH, :] = res.results[c]["o"].T
    return out



# revision 13
# speedup vs baseline: 1.1532x; 1.1532x over previous
"""Trainium2 Bass kernel: dense transformer block (attention + per-batch bmm + FF).

Sharding: 8 cores = (batch b = c//2) x (query-half nh = c%2).
Each core computes attention for all 16 heads over its 1024 query rows
(keys/values over full S=2048, recomputed per batch-pair), then the
per-batch feature-reduction bmm and the feed-forward for its rows.

All heavy matmuls run as float32r (fp22 mantissa, full PE rate at free>=256).
Scores are computed twice: pass 1 feeds a DVE max chain (PSUM-pair fold to
bf16, then bf16 folds at 2x rate) + GPSIMD partition-reduce to get the
per-query row max; pass 2 re-computes scores with a 65th contraction row
(k_aug row = -1, q_aug row = +rowmax) so the shift is baked into PSUM and
ScalarE does shift+exp+evacuate in one activation pass.  V carries a 65th
ones-column so the softmax denominator falls out of the attn@v matmul; z is
normalized by a reciprocal read straight from PSUM, broadcast across
partitions on the idle GPSIMD engine, and one tensor_tensor multiply.
"""

import sys

sys.path.insert(0, "/opt/trn_rl_repo")

import numpy as np

B, S, E, H, HF = 4, 2048, 1024, 16, 64
NH = 1024          # query rows per core
NB = NH // 512     # 512-wide query blocks per core
SCALE = 1.0 / np.sqrt(HF)

_CACHE = {}
_CST = np.concatenate([np.ones((1, S), np.float32), -np.ones((1, S), np.float32)])


def _np_reference(x, attention_mask, Wq, Wk, Wv, Wr, Wff, bff):
    """Fallback (used only if the mask is not all-ones)."""
    x64 = x.astype(np.float64)
    q = np.einsum("bse,hef->bhsf", x64, Wq.astype(np.float64)).reshape(B * H, S, HF)
    k = np.einsum("bse,hef->bhsf", x64, Wk.astype(np.float64)).reshape(B * H, S, HF)
    v = np.einsum("bse,hef->bhsf", x64, Wv.astype(np.float64)).reshape(B * H, S, HF)
    s = np.matmul(q, k.transpose(0, 2, 1))
    s = np.where(attention_mask[0] == 0, -1e9, s)
    s = s * SCALE
    s = s - s.max(axis=-1, keepdims=True)
    p = np.exp(s)
    p /= p.sum(axis=-1, keepdims=True)
    z = np.matmul(p, v).reshape(B, H, S, HF).transpose(0, 2, 1, 3).reshape(B, S, E)
    z = np.matmul(z, Wr.astype(np.float64))
    o = np.maximum(z @ Wff.astype(np.float64).T + bff.astype(np.float64), 0.0)
    return o.astype(np.float32)


def _build():
    import concourse.bacc as bacc
    import concourse.bass as bass
    import concourse.mybir as mybir
    import concourse.tile as tile
    import bass_rust

    F32 = mybir.dt.float32
    BF16 = mybir.dt.bfloat16
    F32R = mybir.dt.float32r
    MULT = mybir.AluOpType.mult
    MAXOP = mybir.AluOpType.max
    EXP = mybir.ActivationFunctionType.Exp
    RELU = mybir.ActivationFunctionType.Relu
    COPY = mybir.ActivationFunctionType.Copy
    RMAX = bass_rust.ReduceOp.max
    PSUM = bass.MemorySpace.PSUM

    def r(ap):
        return ap.bitcast(F32R)

    nc = bacc.Bacc("TRN2", target_bir_lowering=False, debug=False)
    xt_d = nc.dram_tensor("xt", [E, S], F32, kind="ExternalInput")      # x[b].T, cols rolled so my half is first
    wq_d = nc.dram_tensor("wq", [E, E], F32, kind="ExternalInput")      # [e, h*HF+f]
    wk_d = nc.dram_tensor("wk", [E, E], F32, kind="ExternalInput")
    wv_d = nc.dram_tensor("wv", [E, E], F32, kind="ExternalInput")
    wr_d = nc.dram_tensor("wr", [E, E], F32, kind="ExternalInput")      # Wr[b]
    wfft_d = nc.dram_tensor("wfft", [E, E], F32, kind="ExternalInput")  # Wff.T
    bff_d = nc.dram_tensor("bff", [E, 1], F32, kind="ExternalInput")
    cst_d = nc.dram_tensor("cst", [2, S], F32, kind="ExternalInput")    # rows: 1.0, -1.0
    out_d = nc.dram_tensor("o", [E, NH], F32, kind="ExternalOutput")    # [j, n]

    with tile.TileContext(nc) as tc:
        with tc.tile_pool(name="glob", bufs=1) as glob:
            zTn = glob.tile([128, 8, NH], F32)     # normalized z^T: [f-in-pair, echunk, n]
            bfft = glob.tile([128, 8], F32)
            nc.sync.dma_start(out=bfft, in_=bff_d.ap().rearrange("(t p) o -> p (t o)", p=128))

            # ---------------- phase 1: projections + attention ----------------
            with tc.tile_pool(name="p1x", bufs=1) as p1x, \
                 tc.tile_pool(name="wpool", bufs=2) as wpool, \
                 tc.tile_pool(name="qkpool", bufs=2) as qkpool, \
                 tc.tile_pool(name="vpool", bufs=1) as vpool, \
                 tc.tile_pool(name="epool", bufs=3) as epool, \
                 tc.tile_pool(name="spool", bufs=2) as spool, \
                 tc.tile_pool(name="ps_p", bufs=2, space=PSUM) as ps_p, \
                 tc.tile_pool(name="ps_s2", bufs=2, space=PSUM) as ps_s2, \
                 tc.tile_pool(name="ps_z", bufs=2, space=PSUM) as ps_z:

                def load_qk(hp):
                    wq_sb = wpool.tile([128, 8, 128], F32, tag="wq", name="wq_sb")
                    nc.sync.dma_start(out=r(wq_sb), in_=r(wq_d.ap()).rearrange("(i p) c -> p i c", p=128)[:, :, hp * 128:(hp + 1) * 128])
                    wk_sb = wpool.tile([128, 8, 128], F32, tag="wk", name="wk_sb")
                    nc.sync.dma_start(out=r(wk_sb), in_=r(wk_d.ap()).rearrange("(i p) c -> p i c", p=128)[:, :, hp * 128:(hp + 1) * 128])
                    return wq_sb, wk_sb

                wqk_next = load_qk(0)
                xt = p1x.tile([128, 8, S], F32)    # 64KB/part
                for e in range(8):
                    nc.sync.dma_start(out=r(xt[:, e, :]),
                                      in_=r(xt_d.ap()).rearrange("(i p) m -> p i m", p=128)[:, e, :])

                for hp in range(H // 2):           # head pair: heads 2hp, 2hp+1
                    wq_sb, wk_sb = wqk_next

                    # augmented q/k tiles: rows 0:64 = head data (written straight
                    # from PSUM), row 64 = -1 (k) / rowmax (q)
                    qaug = [qkpool.tile([65, NH], F32, tag=f"qaug{i}", name=f"qaug{i}") for i in range(2)]
                    kaug = [qkpool.tile([65, S], F32, tag=f"kaug{i}", name=f"kaug{i}") for i in range(2)]
                    nc.sync.dma_start(out=r(kaug[0][64:65, :]), in_=r(cst_d.ap())[1:2, :])
                    nc.sync.dma_start(out=r(kaug[1][64:65, :]), in_=r(cst_d.ap())[1:2, :])

                    # -------- projections (q on pair ring; k/v on singles ring) --------
                    psq = ps_p.tile([128, 2, 512], F32, tag="pp", name="psq")
                    for j in range(NB):
                        for e in range(8):
                            nc.tensor.matmul(psq[:, j, :], r(wq_sb[:, e, :]), r(xt[:, e, j * 512:(j + 1) * 512]),
                                             start=(e == 0), stop=(e == 7))
                    for j in range(NB):
                        nsl = slice(j * 512, (j + 1) * 512)
                        nc.vector.tensor_copy(out=r(qaug[0][0:64, nsl]), in_=psq[0:64, j, :])
                        nc.scalar.activation(out=r(qaug[1][0:64, nsl]), in_=psq[64:128, j, :], func=COPY)
                    for mb in range(S // 512):
                        psk = ps_s2.tile([128, 512], F32, tag="s2", name="psk")
                        for e in range(8):
                            nc.tensor.matmul(psk, r(wk_sb[:, e, :]), r(xt[:, e, mb * 512:(mb + 1) * 512]),
                                             start=(e == 0), stop=(e == 7))
                        sl = slice(mb * 512, (mb + 1) * 512)
                        nc.vector.tensor_copy(out=r(kaug[0][0:64, sl]), in_=psk[0:64, :])
                        nc.scalar.activation(out=r(kaug[1][0:64, sl]), in_=psk[64:128, :], func=COPY)

                    if hp % 2 == 0:                # v for 4 heads (free dim 256)
                        g = hp // 2
                        wv_sb = wpool.tile([128, 8, 256], F32, tag="wv", bufs=1)
                        nc.sync.dma_start(out=r(wv_sb), in_=r(wv_d.ap()).rearrange("(i p) c -> p i c", p=128)[:, :, g * 256:(g + 1) * 256])
                        v4 = vpool.tile([128, 16, 4, 65], BF16, tag="v4")
                        nc.gpsimd.memset(v4[:, :, :, 64:65], 1.0)
                        for mt in range(16):
                            psv = ps_s2.tile([128, 512], F32, tag="s2", name="psv")
                            for e in range(8):
                                nc.tensor.matmul(psv[:, 0:256], r(xt[:, e, mt * 128:(mt + 1) * 128]), r(wv_sb[:, e, :]),
                                                 start=(e == 0), stop=(e == 7))
                            nc.scalar.copy(out=v4[:, mt, :, 0:64], in_=psv[:, 0:256].rearrange("p (s f) -> p s f", s=4))

                    # -------- pass 1: scores + max chain (both heads, both nb) --------
                    for nb in range(NB):
                        for i in range(2):
                            nsl = slice(nb * 512, (nb + 1) * 512)
                            macc = spool.tile([128, 1024], BF16, tag="macc", name="macc", bufs=4)
                            for pp in range(8):
                                pt = ps_p.tile([128, 2, 512], F32, tag="pp", name="p1t")
                                for j in range(2):
                                    mt = 2 * pp + j
                                    nc.tensor.matmul(pt[:, j, :], r(kaug[i][0:64, mt * 128:(mt + 1) * 128]),
                                                     r(qaug[i][0:64, nsl]), start=True, stop=True)
                                ptf = pt.rearrange("p j f -> p (j f)")
                                if pp == 0:
                                    nc.vector.tensor_copy(out=macc, in_=ptf)
                                else:
                                    nc.vector.tensor_tensor(out=macc, in0=ptf, in1=macc, op=MAXOP)
                            mfold = spool.tile([128, 512], BF16, tag="mfold", name="mfold")
                            nc.vector.tensor_tensor(out=mfold, in0=macc[:, 0:512], in1=macc[:, 512:1024], op=MAXOP)
                            mall = spool.tile([128, 512], F32, tag="mall", name="mall", bufs=4)
                            nc.gpsimd.partition_all_reduce(mall, mfold, channels=128, reduce_op=RMAX)
                            nc.sync.dma_start(out=r(qaug[i][64:65, nsl]), in_=r(mall[0:1, :]))

                    # -------- pass 2 + av (all nb, both heads) --------
                    for nb in range(NB):
                        for i, h in enumerate((2 * hp, 2 * hp + 1)):
                            ka, qa = kaug[i], qaug[i]
                            nsl = slice(nb * 512, (nb + 1) * 512)
                            zps = ps_z.tile([128, 512], F32, tag="zps")
                            g, s4 = h // 4, h % 4
                            for mt in range(16):
                                s2 = ps_s2.tile([128, 512], F32, tag="s2")
                                nc.tensor.matmul(s2, r(ka[0:65, mt * 128:(mt + 1) * 128]), r(qa[0:65, nsl]),
                                                 start=True, stop=True)
                                expt = epool.tile([128, 512], BF16, tag="expt")
                                nc.scalar.activation(out=expt, in_=s2, func=EXP, scale=float(SCALE))
                                nc.tensor.matmul(zps[0:65, :], v4[:, mt, s4, :], expt,
                                                 start=(mt == 0), stop=(mt == 15))
                            # normalize: reciprocal straight from PSUM, broadcast on
                            # GPSIMD, single TT multiply into zTn
                            rec = spool.tile([1, 512], F32, tag="rec", name="rec")
                            with nc.allow_low_precision(reason="recip feeds a bf16-grade normalize"):
                                nc.vector.reciprocal(out=rec, in_=zps[64:65, :])
                            recb = spool.tile([64, 512], F32, tag="recb", name="recb")
                            nc.gpsimd.partition_broadcast(recb, rec, channels=64)
                            ec = h // 2
                            po = 64 * (h % 2)
                            nc.vector.tensor_tensor(out=r(zTn[po:po + 64, ec, nsl]), in0=zps[0:64, :],
                                                    in1=recb, op=MULT)

                    if hp + 1 < H // 2:
                        wqk_next = load_qk(hp + 1)

            # ---------------- phase 2: y^T = Wr^T-style bmm, then FF ----------------
            with tc.tile_pool(name="p2w", bufs=1) as p2w, \
                 tc.tile_pool(name="p2y", bufs=1) as p2y, \
                 tc.tile_pool(name="p2o", bufs=2) as p2o, \
                 tc.tile_pool(name="ps_y", bufs=3, space=PSUM) as ps_y:
                wr_sb = p2w.tile([128, 8, E], F32, tag="wr")
                nc.sync.dma_start(out=r(wr_sb), in_=r(wr_d.ap()).rearrange("(i p) c -> p i c", p=128))
                wff_sb = p2w.tile([128, 8, E], F32, tag="wff")
                nc.sync.dma_start(out=r(wff_sb), in_=r(wfft_d.ap()).rearrange("(i p) c -> p i c", p=128))
                yT = p2y.tile([128, 8, NH], F32)
                for dt in range(8):
                    psy = ps_y.tile([128, NH], F32, tag="psy")
                    for ec in range(8):
                        for half in range(2):
                            nc.tensor.matmul(psy[:, half * 512:(half + 1) * 512],
                                             r(wr_sb[:, ec, dt * 128:(dt + 1) * 128]),
                                             r(zTn[:, ec, half * 512:(half + 1) * 512]),
                                             start=(ec == 0), stop=(ec == 7))
                    if dt % 2 == 0:
                        nc.vector.tensor_copy(out=r(yT[:, dt, :]), in_=psy)
                    else:
                        nc.scalar.activation(out=r(yT[:, dt, :]), in_=psy, func=COPY)
                for jt in range(8):
                    pso = ps_y.tile([128, NH], F32, tag="psy")
                    for dc in range(8):
                        for half in range(2):
                            nc.tensor.matmul(pso[:, half * 512:(half + 1) * 512],
                                             r(wff_sb[:, dc, jt * 128:(jt + 1) * 128]),
                                             r(yT[:, dc, half * 512:(half + 1) * 512]),
                                             start=(dc == 0), stop=(dc == 7))
                    ot = p2o.tile([128, NH], F32, tag="ot")
                    nc.scalar.activation(out=ot, in_=pso, func=RELU, bias=bfft[:, jt:jt + 1], scale=1.0)
                    nc.sync.dma_start(out=out_d.ap()[jt * 128:(jt + 1) * 128, :], in_=ot)

    nc.compile()
    return nc


def _get_module():
    if "nc" not in _CACHE:
        _CACHE["nc"] = _build()
    return _CACHE["nc"]


def kernel(x, attention_mask, Wq, Wk, Wv, Wr, Wff, bff):
    from concourse import bass_utils

    x = np.asarray(x, dtype=np.float32)
    attention_mask = np.asarray(attention_mask)
    Wq = np.asarray(Wq, dtype=np.float32)
    Wk = np.asarray(Wk, dtype=np.float32)
    Wv = np.asarray(Wv, dtype=np.float32)
    Wr = np.asarray(Wr, dtype=np.float32)
    Wff = np.asarray(Wff, dtype=np.float32)
    bff = np.asarray(bff, dtype=np.float32)

    if not np.all(attention_mask == 1):
        return _np_reference(x, attention_mask, Wq, Wk, Wv, Wr, Wff, bff)

    nc = _get_module()
    wq2 = np.ascontiguousarray(Wq.transpose(1, 0, 2).reshape(E, E))
    wk2 = np.ascontiguousarray(Wk.transpose(1, 0, 2).reshape(E, E))
    wv2 = np.ascontiguousarray(Wv.transpose(1, 0, 2).reshape(E, E))
    wfft = np.ascontiguousarray(Wff.T)
    bff2 = np.ascontiguousarray(bff.reshape(E, 1))

    in_maps = []
    for c in range(8):
        b, nh = c // 2, c % 2
        xt = x[b].T
        if nh:
            xt = np.concatenate([xt[:, NH:], xt[:, :NH]], axis=1)
        in_maps.append({
            "xt": np.ascontiguousarray(xt),
            "wq": wq2, "wk": wk2, "wv": wv2,
            "wr": np.ascontiguousarray(Wr[b]),
            "wfft": wfft, "bff": bff2,
            "cst": _CST,
        })

    res = bass_utils.run_bass_kernel_spmd(nc, in_maps, core_ids=list(range(8)), **_CACHE.get("run_kwargs", {}))
    _CACHE["last_result"] = res

    out = np.empty((B, S, E), dtype=np.float32)
    for c in range(8):
        b, nh = c // 2, c % 2
        out[b, nh * NH:(nh + 1) * NH, :] = res.results[c]["o"].T
    return out
